# revision 9
# baseline (speedup 1.0000x reference)
"""DEDICOM decoder edge scoring on 8 TRN2 NeuronCores.

scores[e] = (z[src_e] * d) @ R @ (z[dst_e] * d)  for 1M edges.

Strategy (data-parallel over edges, z replicated, all tables bf16):
  - host passes z (bf16, row layout) and z^T (bf16); device builds
    M = (d (x) d) * R and the table Y = z @ M in bf16.  Y is stored
    BLOCKED in DRAM as [128, nblk, 128]: row r lives at partition
    r % 128, block-column r // 128 — so each 128-row build batch is one
    contiguous [:, c, :] slice (trivial DMA) and a gather still sees a
    flat [nblk*128, 128] row table via a rearranged AP with
    host-permuted indices  idx(r) = (r % 128) * nblk + r // 128.
  - per CHUNK-edge chunk: dma_gather Y[src] rows and z[dst] rows (bf16,
    256 B rows) round-robin over 4 SWDGE queues.  Queue q runs on Q7
    core pair q, so descriptor generation for up to 4 gathers pipelines
    (~3x the single-queue rate, which is the kernel's bottleneck).
  - fused DVE multiply + X-axis reduce gives per-edge dot products.
  - dma_gather indices are int16, so tables are split in two halves
    (rows < 32000 and >= 32000); the host buckets each core's edges by
    (src half, dst half) and un-permutes the scores afterwards.
"""
import numpy as np
import ml_dtypes
import concourse.bacc as bacc
import concourse.mybir as mybir
from concourse.tile import TileContext
from concourse.bass_utils import run_bass_kernel_spmd

N_CORES = 8
N_NODES = 50000
D = 128
HALF = 32000          # int16-safe table split point
CHUNK = 4096          # edges per dma_gather call
NQ = 4                # SWDGE queues (queue q -> Q7 core pair q)
NBLK0 = HALF // 128                       # 250
NBLK1 = (N_NODES - HALF + 127) // 128     # 141
NPAD = (NBLK0 + NBLK1) * 128              # 50048


def _build_program(nchunks_per_bucket):
    total_chunks = sum(nchunks_per_bucket)
    ntot = total_chunks * CHUNK
    nc = bacc.Bacc("TRN2", num_devices=N_CORES, num_swdge_queues=NQ)
    zb = nc.declare_dram_parameter("zb", [N_NODES, D], mybir.dt.bfloat16, isOutput=False)
    zbT = nc.declare_dram_parameter("zbT", [128, NPAD], mybir.dt.bfloat16, isOutput=False)
    R = nc.declare_dram_parameter("R", [D, D], mybir.dt.float32, isOutput=False)
    dr = nc.declare_dram_parameter("dr", [1, D], mybir.dt.float32, isOutput=False)
    isrc = nc.declare_dram_parameter("isrc", [128, ntot // 16], mybir.dt.int16, isOutput=False)
    idst = nc.declare_dram_parameter("idst", [128, ntot // 16], mybir.dt.int16, isOutput=False)
    scores = nc.declare_dram_parameter("scores", [128, ntot // 128], mybir.dt.float32, isOutput=True)
    Y0 = nc.dram_tensor("Y0", [128, NBLK0, 128], mybir.dt.bfloat16)
    Y1 = nc.dram_tensor("Y1", [128, NBLK1, 128], mybir.dt.bfloat16)

    with TileContext(nc) as tc:
        with (
            tc.tile_pool(name="const", bufs=1) as constp,
            tc.tile_pool(name="mps", bufs=1, space="PSUM") as mpsp,
            tc.tile_pool(name="zslab", bufs=2) as zslabp,
            tc.tile_pool(name="ybuild", bufs=3, space="PSUM") as ybps,
            tc.tile_pool(name="ysb", bufs=3) as ysbp,
            tc.tile_pool(name="idxp", bufs=1) as idxp,
            tc.tile_pool(name="gsrc", bufs=4) as gsrcp,
            tc.tile_pool(name="gdst", bufs=8) as gdstp,
            tc.tile_pool(name="prod", bufs=2) as prodp,
            tc.tile_pool(name="scorep", bufs=1) as scorep,
        ):
            # ---- constants: R, d_r, M = (d (x) d) * R in bf16 ----
            R_sb = constp.tile([128, D], mybir.dt.float32)
            nc.sync.dma_start(out=R_sb[:], in_=R[:])
            dr_sb = constp.tile([1, D], mybir.dt.float32)
            nc.sync.dma_start(out=dr_sb[:], in_=dr[:])
            DRps = mpsp.tile([128, 128], mybir.dt.float32)
            nc.tensor.matmul(out=DRps[:], lhsT=dr_sb[:], rhs=dr_sb[:], start=True, stop=True)
            Mb = constp.tile([128, D], mybir.dt.bfloat16)
            nc.vector.tensor_tensor(out=Mb[:], in0=R_sb[:], in1=DRps[:], op=mybir.AluOpType.mult)

            # ---- indices (load early so gathers aren't blocked on them) ----
            isrc_sb = idxp.tile([128, ntot // 16], mybir.dt.int16)
            nc.sync.dma_start(out=isrc_sb[:], in_=isrc[:])
            idst_sb = idxp.tile([128, ntot // 16], mybir.dt.int16)
            nc.sync.dma_start(out=idst_sb[:], in_=idst[:])

            # ---- build Y = z @ M (bf16, blocked layout), Y0 then Y1 ----
            def build_y(Yt, nblk, blk_base):
                c0 = blk_base * 128
                for s0 in range(0, nblk, 32):          # 32-block slabs
                    sw = min(32, nblk - s0)
                    zsb = zslabp.tile([128, 32 * 128], mybir.dt.bfloat16, tag="zslab")
                    nc.sync.dma_start(
                        out=zsb[:, :sw * 128],
                        in_=zbT[:, c0 + s0 * 128:c0 + (s0 + sw) * 128])
                    i = 0
                    while i < sw:
                        w = min(4, sw - i)
                        yps = ybps.tile([128, 4, 128], mybir.dt.float32, tag="yps")
                        for j in range(w):
                            nc.tensor.matmul(
                                out=yps[:, j, :],
                                lhsT=zsb[:, (i + j) * 128:(i + j + 1) * 128],
                                rhs=Mb[:], start=True, stop=True)
                        ysb = ysbp.tile([128, 4, 128], mybir.dt.bfloat16, tag="ysb")
                        nc.scalar.copy(out=ysb[:, :w, :], in_=yps[:, :w, :])
                        nc.sync.dma_start(out=Yt[:, s0 + i:s0 + i + w, :], in_=ysb[:, :w, :])
                        i += w

            build_y(Y0, NBLK0, 0)
            build_y(Y1, NBLK1, NBLK0)
            Y0f = Y0[:, :, :].rearrange("p c f -> (p c) f")
            Y1f = Y1[:, :, :].rearrange("p c f -> (p c) f")

            # ---- main loop: software-pipelined gather + fused dot ----
            # dst gathers (no Y dependency) run PRE chunks ahead of src
            # gathers so the Y0 build overlaps the first dst gathers.
            score_sb = scorep.tile([128, ntot // 128], mybir.dt.float32)
            chunks = []          # (src_table, dst_table) per chunk, in order
            for b in range(4):
                src_t = Y0f if b < 2 else Y1f
                dst_t = zb[:, :] if b % 2 == 0 else zb[HALF:, :]
                chunks.extend([(src_t, dst_t)] * nchunks_per_bucket[b])
            nchunks = len(chunks)
            PRE = 4
            gi = 0               # global gather counter -> queue cycling
            g2_tiles = {}

            def emit_dst(k):
                nonlocal gi
                c16 = k * (CHUNK // 16)
                g2 = gdstp.tile([128, CHUNK // 128, D], mybir.dt.bfloat16, tag="g2")
                nc.gpsimd.dma_gather(
                    g2[:], chunks[k][1], idst_sb[:, c16:c16 + CHUNK // 16],
                    CHUNK, CHUNK, D, single_packet=False, queue_num=gi % NQ)
                gi += 1
                g2_tiles[k] = g2

            for k in range(min(PRE, nchunks)):
                emit_dst(k)
            for k in range(nchunks):
                c16 = k * (CHUNK // 16)
                g1 = gsrcp.tile([128, CHUNK // 128, D], mybir.dt.bfloat16, tag="g1")
                nc.gpsimd.dma_gather(
                    g1[:], chunks[k][0], isrc_sb[:, c16:c16 + CHUNK // 16],
                    CHUNK, CHUNK, D, single_packet=False, queue_num=gi % NQ)
                gi += 1
                if k + PRE < nchunks:
                    emit_dst(k + PRE)
                g2 = g2_tiles.pop(k)
                prod = prodp.tile([128, CHUNK // 128, D], mybir.dt.bfloat16, tag="prod")
                nc.vector.tensor_tensor(
                    out=prod[:], in0=g1[:], in1=g2[:], op=mybir.AluOpType.mult)
                nc.vector.tensor_reduce(
                    out=score_sb[:, k * (CHUNK // 128):(k + 1) * (CHUNK // 128)],
                    in_=prod[:], axis=mybir.AxisListType.X, op=mybir.AluOpType.add)
            nc.sync.dma_start(out=scores[:], in_=score_sb[:])
    nc.compile()
    return nc


def _blocked_idx(r, nblk):
    # row r of a [128, nblk, 128] blocked table -> flat gather index
    return (r % 128) * nblk + r // 128


def _prepare(inputs):
    z = np.asarray(inputs["z"], dtype=np.float32)
    R = np.ascontiguousarray(np.asarray(inputs["R"], dtype=np.float32))
    Dm = np.asarray(inputs["D"], dtype=np.float32)
    edge_index = np.asarray(inputs["edge_index"])
    rel = int(np.asarray(inputs["relation_idx"]))
    dr = np.ascontiguousarray(Dm[rel:rel + 1, :])
    zb = np.ascontiguousarray(z.astype(ml_dtypes.bfloat16))
    zbT = np.zeros((128, NPAD), dtype=ml_dtypes.bfloat16)
    zbT[:, :N_NODES] = zb.T

    B = edge_index.shape[1]
    assert B % N_CORES == 0
    per = B // N_CORES
    src_all = edge_index[0].astype(np.int64)
    dst_all = edge_index[1].astype(np.int64)

    cores = []
    counts = np.zeros((N_CORES, 4), np.int64)
    for c in range(N_CORES):
        s = src_all[c * per:(c + 1) * per]
        d = dst_all[c * per:(c + 1) * per]
        bkey = (s >= HALF).astype(np.int64) * 2 + (d >= HALF).astype(np.int64)
        order = np.argsort(bkey, kind="stable")
        cores.append((s[order], d[order], order))
        counts[c] = np.bincount(bkey, minlength=4)
    nch = [int(np.ceil(counts[:, b].max() / CHUNK)) for b in range(4)]
    ntot = sum(nch) * CHUNK

    def wrap(a):
        w = np.ascontiguousarray(a.reshape(-1, 16).T.astype(np.int16))
        return np.tile(w, (8, 1))

    in_maps = []
    for c in range(N_CORES):
        ssorted, dsorted, _ = cores[c]
        sarr = np.zeros(ntot, np.int64)
        darr = np.zeros(ntot, np.int64)
        off_in = 0
        off_out = 0
        for b in range(4):
            n = int(counts[c, b])
            s_loc = ssorted[off_in:off_in + n] - (HALF if b >= 2 else 0)
            sarr[off_out:off_out + n] = _blocked_idx(s_loc, NBLK0 if b < 2 else NBLK1)
            darr[off_out:off_out + n] = dsorted[off_in:off_in + n] - (HALF if b % 2 else 0)
            off_in += n
            off_out += nch[b] * CHUNK
        in_maps.append({"zb": zb, "zbT": zbT, "R": R, "dr": dr,
                        "isrc": wrap(sarr), "idst": wrap(darr)})
    return in_maps, cores, counts, nch, ntot, per, B


def _collect(res, cores, counts, nch, ntot, per, B):
    out = np.empty(B, np.float32)
    nchunks = ntot // CHUNK
    epp = CHUNK // 128      # edges per partition per chunk
    for c in range(N_CORES):
        sc = np.asarray(res.results[c]["scores"])  # [128, ntot//128]
        padded = sc.reshape(128, nchunks, epp).transpose(1, 2, 0).reshape(-1)
        _, _, order = cores[c]
        vals = np.empty(per, np.float32)
        off_in = 0
        off_out = 0
        for b in range(4):
            n = int(counts[c, b])
            vals[off_in:off_in + n] = padded[off_out:off_out + n]
            off_in += n
            off_out += nch[b] * CHUNK
        outslice = np.empty(per, np.float32)
        outslice[order] = vals
        out[c * per:(c + 1) * per] = outslice
    return out


def kernel_with_time(inputs, trace=False):
    in_maps, cores, counts, nch, ntot, per, B = _prepare(inputs)
    nc = _build_program(nch)
    res = run_bass_kernel_spmd(nc, in_maps, list(range(N_CORES)), trace=trace)
    out = _collect(res, cores, counts, nch, ntot, per, B)
    return out, res.exec_time_ns, res


def kernel(**inputs):
    out, _, _ = kernel_with_time(inputs, trace=False)
    return out
